# revision 1
# baseline (speedup 1.0000x reference)
"""MetaNodeRNN Trainium2 kernel.

FNN (2-layer MLP) -> 512-step GRU scan -> output Linear, data-parallel over
batch across 8 NeuronCores (32 rows/core). GRU state is kept "transposed"
(H on partitions, batch on free dim) so every per-step tensor op uses all
128 partitions and no transposes are needed inside the scan.

Key tricks:
  - For steps >= 2 the GRU input equals the hidden state, so the r/z gate
    matmuls use merged weights (w_ih + w_hh)[:2H] -- one matmul instead of two.
  - Biases are injected into the PSUM accumulation groups via tiny one-hot
    matmuls (K=4 / K=1), so activations never need per-partition bias vectors.
  - Weights are the stationary matmul operand (bf16 => fast weight load);
    the state (bf16) streams through.
  - y_t = h_t @ out_w.T + out_b is computed interleaved on the PE,
    accumulated 4 steps per PSUM bank, then copied to SBUF + DMA'd out.
"""

import os

import ml_dtypes
import numpy as np

B, E, H, M, T_FULL = 256, 256, 256, 128, 512
NC = 8
BS = B // NC          # 32 batch rows per core
KC = H // 128         # 2 contraction chunks of 128
YB = 4                # y steps accumulated per PSUM bank

BF16 = ml_dtypes.bfloat16


# ---------------------------------------------------------------------------
# walrus in this container rejects >1 semaphore wait per CTRL instruction;
# redistribute excess waits onto inserted Drain instructions.
# ---------------------------------------------------------------------------
def _split_excess_waits(nc, max_waits=1):
    import concourse.mybir as mybir

    n_fixed = 0
    for bb in nc.main_func.blocks:
        insts = list(bb.instructions)
        out = []
        changed = False
        for ins in insts:
            si = ins.sync_info
            if si is not None and si.on_wait and len(si.on_wait) > max_waits:
                waits = list(si.on_wait)
                extra, keep = waits[:-max_waits], waits[-max_waits:]
                k = 0
                while extra:
                    chunk, extra = extra[:max_waits], extra[max_waits:]
                    nop = mybir.InstDrain(
                        name=f"{ins.name}-waitsplit-{k}",
                        engine=ins.engine,
                        ins=[],
                        outs=[],
                        sync_info=mybir.SyncInfo(on_wait=chunk, on_update=[]),
                    )
                    out.append(nop)
                    k += 1
                si.on_wait = keep
                ins.sync_info = si
                changed = True
                n_fixed += 1
            out.append(ins)
        if changed:
            try:
                bb.instructions = out
            except Exception:
                bb.instructions.clear()
                bb.instructions.extend(out)
    return n_fixed


# ---------------------------------------------------------------------------
# bass program
# ---------------------------------------------------------------------------
def _build_nc(T):
    import concourse.bass as bass
    import concourse.mybir as mybir
    from concourse.tile import TileContext

    dt = mybir.dt
    AF = mybir.ActivationFunctionType

    nc = bass.Bass()

    def din(name, shape, dtype=dt.bfloat16):
        return nc.dram_tensor(name, shape, dtype, kind="ExternalInput")

    # weights, pre-chunked on host into lhsT layout [128(k-rows), m, k, 128(cols)]
    wrz = din("wrz", [128, 4, KC, 128])          # merged (w_ih+w_hh)[:2H], steps>=2
    whr = din("whr", [128, 4, KC, 128])          # w_hh[:2H], step 1
    wni = din("wni", [128, 2, KC, 128])          # w_ih[2H:3H]
    wnh = din("wnh", [128, 2, KC, 128])          # w_hh[2H:3H]
    w1 = din("w1", [128, 2, KC, 128])            # fnn_w1
    w2 = din("w2", [128, 2, KC, 128])            # fnn_w2
    owt = din("owt", [128, KC, 128])             # out_w.T chunks
    embT = din("embT", [128, KC, BS])            # emb shard, host-transposed

    brz_l = din("brz_l", [4, 128])               # (b_ih+b_hh)[:2H] rows per m-tile
    bh_l = din("bh_l", [4, 128])                 # b_hh[:2H] rows (step 1)
    bni_l = din("bni_l", [2, 128])               # b_ih[2H:]
    bnh_l = din("bnh_l", [2, 128])               # b_hh[2H:]
    b1_l = din("b1_l", [2, 128])                 # fnn_b1
    b2_l = din("b2_l", [2, 128])                 # fnn_b2
    sel4 = din("sel4", [4, 4, BS])               # one-hot bias spreaders
    sel2 = din("sel2", [2, 2, BS])
    ones1 = din("ones1", [1, BS])
    oby = din("oby", [1, YB, M])                 # out_b tiled for the y bank

    y_out = nc.dram_tensor("y", [BS, T, M], dt.float32, kind="ExternalOutput")
    hx_out = nc.dram_tensor("hxT", [128, KC, BS], dt.float32, kind="ExternalOutput")

    with TileContext(nc) as tc:
        with (
            tc.tile_pool(name="wpool", bufs=1) as wp,
            tc.tile_pool(name="state", bufs=3) as sp,
            tc.tile_pool(name="tmp", bufs=3) as tp,
            tc.tile_pool(name="ysb", bufs=3) as yp,
            tc.tile_pool(name="ps_rz", bufs=2, space="PSUM") as prz,
            tc.tile_pool(name="ps_hn", bufs=2, space="PSUM") as phn,
            tc.tile_pool(name="ps_in", bufs=2, space="PSUM") as pin,
            tc.tile_pool(name="ps_y", bufs=2, space="PSUM") as py,
        ):
            def load(dram, shape, tag):
                t = wp.tile(shape, dram.dtype, tag=tag)
                nc.sync.dma_start(t[:], dram[:])
                return t

            wrz_s = load(wrz, [128, 4, KC, 128], "wrz")
            whr_s = load(whr, [128, 4, KC, 128], "whr")
            wni_s = load(wni, [128, 2, KC, 128], "wni")
            wnh_s = load(wnh, [128, 2, KC, 128], "wnh")
            w1_s = load(w1, [128, 2, KC, 128], "w1")
            w2_s = load(w2, [128, 2, KC, 128], "w2")
            owt_s = load(owt, [128, KC, 128], "owt")
            embT_s = load(embT, [128, KC, BS], "embT")
            brz_s = load(brz_l, [4, 128], "brz")
            bh_s = load(bh_l, [4, 128], "bh")
            bni_s = load(bni_l, [2, 128], "bni")
            bnh_s = load(bnh_l, [2, 128], "bnh")
            b1_s = load(b1_l, [2, 128], "b1")
            b2_s = load(b2_l, [2, 128], "b2")
            sel4_s = load(sel4, [4, 4, BS], "sel4")
            sel2_s = load(sel2, [2, 2, BS], "sel2")
            ones_s = load(ones1, [1, BS], "ones1")
            oby_s = load(oby, [1, YB, M], "oby")

            MMs = nc.tensor.matmul

            # ---------------- FNN prologue ----------------
            # f1 = relu(W1 @ embT + b1)   (bias via one-hot K=2 matmul)
            f1_ps = prz.tile([128, 4, BS], dt.float32, tag="rzps")
            MMs(f1_ps[:, 0:2, :], b1_s[:], sel2_s[:], start=True, stop=False,
                skip_group_check=True)
            for m in range(2):
                for k in range(KC):
                    MMs(f1_ps[:, m, :], w1_s[:, m, k, :], embT_s[:, k, :],
                        start=False, stop=(k == KC - 1), skip_group_check=True)
            f1_sb = tp.tile([128, KC, BS], dt.bfloat16, tag="f1")
            nc.scalar.activation(f1_sb[:], f1_ps[:, 0:2, :], AF.Relu)

            # hx = W2 @ f1 + b2
            hx_ps = phn.tile([128, 2, BS], dt.float32, tag="hnps")
            MMs(hx_ps[:], b2_s[:], sel2_s[:], start=True, stop=False,
                skip_group_check=True)
            for m in range(2):
                for k in range(KC):
                    MMs(hx_ps[:, m, :], w2_s[:, m, k, :], f1_sb[:, k, :],
                        start=False, stop=(k == KC - 1), skip_group_check=True)
            h_prev = sp.tile([128, KC, BS], dt.bfloat16, tag="h")
            nc.scalar.activation(h_prev[:], hx_ps[:], AF.Copy)

            # ---------------- GRU scan ----------------
            y_ps = None
            for t in range(T):
                first = t == 0
                w_rz = whr_s if first else wrz_s
                b_rz = bh_s if first else brz_s

                # r/z gates: psum bank [128, (r0,r1,z0,z1), BS]
                rz_ps = prz.tile([128, 4, BS], dt.float32, tag="rzps")
                MMs(rz_ps[:], b_rz[:], sel4_s[:], start=True, stop=False,
                    skip_group_check=True)
                # r tiles first (they head the critical chain)
                for m in (0, 1, 2, 3):
                    for k in range(KC):
                        MMs(rz_ps[:, m, :], w_rz[:, m, k, :], h_prev[:, k, :],
                            start=False, stop=(k == KC - 1),
                            skip_group_check=True)
                    if m == 1:
                        # hn-path matmuls fill the PE while sigmoid(r) runs
                        hn_ps = phn.tile([128, 2, BS], dt.float32, tag="hnps")
                        MMs(hn_ps[:], bnh_s[:], sel2_s[:], start=True,
                            stop=False, skip_group_check=True)
                        for mm in range(2):
                            for k in range(KC):
                                MMs(hn_ps[:, mm, :], wnh_s[:, mm, k, :],
                                    h_prev[:, k, :], start=False,
                                    stop=(k == KC - 1), skip_group_check=True)
                        r_sb = tp.tile([128, 2, BS], dt.bfloat16, tag="r")
                        nc.scalar.activation(r_sb[:], rz_ps[:, 0:2, :],
                                             AF.Sigmoid)

                # i_n path (bias always; matmuls only for steps >= 2)
                in_ps = pin.tile([128, 2, BS], dt.float32, tag="inps")
                MMs(in_ps[:], bni_s[:], sel2_s[:], start=True, stop=first,
                    skip_group_check=True)
                if not first:
                    for mm in range(2):
                        for k in range(KC):
                            MMs(in_ps[:, mm, :], wni_s[:, mm, k, :],
                                h_prev[:, k, :], start=False,
                                stop=(k == KC - 1), skip_group_check=True)

                z_sb = tp.tile([128, 2, BS], dt.bfloat16, tag="z")
                nc.scalar.activation(z_sb[:], rz_ps[:, 2:4, :], AF.Sigmoid)

                # n = tanh(i_n + r * h_n)
                rhn = tp.tile([128, 2, BS], dt.float32, tag="rhn")
                nc.vector.tensor_mul(rhn[:], hn_ps[:], r_sb[:])
                npre = tp.tile([128, 2, BS], dt.float32, tag="npre")
                nc.vector.tensor_add(npre[:], in_ps[:], rhn[:])
                n_sb = tp.tile([128, 2, BS], dt.bfloat16, tag="n")
                nc.scalar.activation(n_sb[:], npre[:], AF.Tanh)

                # h' = n + z * (h - n)
                hmn = tp.tile([128, 2, BS], dt.bfloat16, tag="hmn")
                nc.vector.tensor_sub(hmn[:], h_prev[:], n_sb[:])
                zm = tp.tile([128, 2, BS], dt.bfloat16, tag="zm")
                nc.vector.tensor_mul(zm[:], z_sb[:], hmn[:])
                h_new = sp.tile([128, KC, BS], dt.bfloat16, tag="h")
                nc.vector.tensor_add(h_new[:], n_sb[:], zm[:])

                # y_t = h' @ out_w.T + out_b  (batch-major psum [BS, M])
                j = t % YB
                if j == 0:
                    y_ps = py.tile([BS, YB, M], dt.float32, tag="yps")
                    MMs(y_ps[:], ones_s[:], oby_s[:], start=True, stop=False,
                        skip_group_check=True)
                for k in range(KC):
                    MMs(y_ps[:, j, :], h_new[:, k, :], owt_s[:, k, :],
                        start=False, stop=(k == KC - 1), skip_group_check=True)
                if j == YB - 1 or t == T - 1:
                    y_sb = yp.tile([BS, YB, M], dt.float32, tag="ysb")
                    nc.scalar.activation(y_sb[:, : j + 1, :], y_ps[:, : j + 1, :],
                                         AF.Copy)
                    t0 = t - j
                    nc.sync.dma_start(y_out[:, t0 : t + 1, :], y_sb[:, : j + 1, :])

                h_prev = h_new

            # hx_final (still transposed; host untransposes)
            hxf = tp.tile([128, KC, BS], dt.float32, tag="hxf")
            nc.scalar.activation(hxf[:], h_prev[:], AF.Copy)
            nc.sync.dma_start(hx_out[:], hxf[:])

    _split_excess_waits(nc)
    return nc


# ---------------------------------------------------------------------------
# host side
# ---------------------------------------------------------------------------
def _chunk_lhsT(w, m_tiles):
    """[m_tiles*128, KC*128] weight -> lhsT chunks [128(k-rows), m, k, 128(m-cols)].

    lhsT for (m, k) must be W[m*128:(m+1)*128, k*128:(k+1)*128].T
    """
    m4 = w.reshape(m_tiles, 128, KC, 128)        # [m, c(row), k, p(col of chunk)]
    return np.ascontiguousarray(m4.transpose(3, 0, 2, 1)).astype(BF16)


def kernel(emb, fnn_w1, fnn_b1, fnn_w2, fnn_b2,
           gru_w_ih, gru_b_ih, gru_w_hh, gru_b_hh, out_w, out_b):
    from concourse.bass_utils import run_bass_kernel_spmd

    T = int(os.environ.get("MNR_T", T_FULL))

    emb = np.asarray(emb, np.float32)
    w_ih = np.asarray(gru_w_ih, np.float32)
    w_hh = np.asarray(gru_w_hh, np.float32)
    b_ih = np.asarray(gru_b_ih, np.float32)
    b_hh = np.asarray(gru_b_hh, np.float32)

    common = {
        "wrz": _chunk_lhsT(w_ih[: 2 * H] + w_hh[: 2 * H], 4),
        "whr": _chunk_lhsT(w_hh[: 2 * H], 4),
        "wni": _chunk_lhsT(w_ih[2 * H :], 2),
        "wnh": _chunk_lhsT(w_hh[2 * H :], 2),
        "w1": _chunk_lhsT(np.asarray(fnn_w1, np.float32), 2),
        "w2": _chunk_lhsT(np.asarray(fnn_w2, np.float32), 2),
        "owt": np.ascontiguousarray(
            np.asarray(out_w, np.float32).reshape(M, KC, 128).transpose(2, 1, 0)
        ).astype(BF16),
        "brz_l": (b_ih[: 2 * H] + b_hh[: 2 * H]).reshape(4, 128).astype(BF16),
        "bh_l": b_hh[: 2 * H].reshape(4, 128).astype(BF16),
        "bni_l": b_ih[2 * H :].reshape(2, 128).astype(BF16),
        "bnh_l": b_hh[2 * H :].reshape(2, 128).astype(BF16),
        "b1_l": np.asarray(fnn_b1, np.float32).reshape(2, 128).astype(BF16),
        "b2_l": np.asarray(fnn_b2, np.float32).reshape(2, 128).astype(BF16),
        "sel4": np.repeat(np.eye(4, dtype=np.float32)[:, :, None], BS, 2).astype(BF16),
        "sel2": np.repeat(np.eye(2, dtype=np.float32)[:, :, None], BS, 2).astype(BF16),
        "ones1": np.ones((1, BS), BF16),
        "oby": np.broadcast_to(
            np.asarray(out_b, np.float32), (1, YB, M)
        ).astype(BF16).copy(),
    }

    in_maps = []
    for c in range(NC):
        sh = emb[c * BS : (c + 1) * BS]                       # [BS, E]
        embT = np.ascontiguousarray(
            sh.T.reshape(KC, 128, BS).transpose(1, 0, 2)
        ).astype(BF16)
        m = dict(common)
        m["embT"] = embT
        in_maps.append(m)

    nc = _build_nc(T)
    trace = bool(int(os.environ.get("MNR_TRACE", "0")))
    res = run_bass_kernel_spmd(
        nc, in_maps, core_ids=list(range(NC)), trace=trace,
    )
    kernel.last_result = res

    y = np.empty((B, T, M), np.float32)
    hx = np.empty((B, H), np.float32)
    for c in range(NC):
        r = res.results[c]
        y[c * BS : (c + 1) * BS] = r["y"]
        hxT = r["hxT"]                                        # [128, KC, BS]
        hx[c * BS : (c + 1) * BS] = hxT.transpose(1, 0, 2).reshape(H, BS).T
    return y, hx


# revision 6
# speedup vs baseline: 13.8303x; 13.8303x over previous
"""MetaNodeRNN Trainium2 kernel.

FNN (2-layer MLP) -> 512-step GRU scan -> output Linear, data-parallel over
batch across 8 NeuronCores (32 rows/core). GRU state is kept "transposed"
(H on partitions, batch on free dim) so every per-step tensor op uses all
128 partitions and no transposes are needed inside the scan.

Key tricks:
  - For steps >= 2 the GRU input equals the hidden state, so the r/z gate
    matmuls use merged weights (w_ih + w_hh)[:2H] -- one matmul instead of two.
  - Biases are injected into the PSUM accumulation groups via tiny one-hot
    matmuls (K=4 / K=2 / K=1), so activations never need bias vectors.
  - Weights are the stationary matmul operand (bf16 => fast weight load);
    the state (bf16) streams through.
  - y_t = h_t @ out_w.T + out_b is computed interleaved on the PE,
    accumulated 4 steps per PSUM bank, then copied to SBUF + DMA'd out.

MNR_T overrides the step count (testing); MNR_REPEAT wraps the whole
FNN+scan+output body in a hardware For_i loop (timing calibration).
"""

import os

import ml_dtypes
import numpy as np

B, E, H, M, T_FULL = 256, 256, 256, 128, 512
NC = 8
BS = B // NC          # 32 batch rows per core
KC = H // 128         # 2 contraction chunks of 128
YB = 4                # y steps accumulated per PSUM bank

BF16 = ml_dtypes.bfloat16


# ---------------------------------------------------------------------------
# walrus in this container rejects >1 semaphore wait per CTRL instruction;
# redistribute excess waits onto inserted Drain instructions.
# ---------------------------------------------------------------------------
def _split_excess_waits(nc, max_waits=1):
    import concourse.mybir as mybir

    n_fixed = 0
    for bb in nc.main_func.blocks:
        insts = list(bb.instructions)
        out = []
        changed = False
        for ins in insts:
            si = ins.sync_info
            if si is not None and si.on_wait and len(si.on_wait) > max_waits:
                waits = list(si.on_wait)
                extra, keep = waits[:-max_waits], waits[-max_waits:]
                k = 0
                while extra:
                    chunk, extra = extra[:max_waits], extra[max_waits:]
                    nop = mybir.InstDrain(
                        name=f"{ins.name}-waitsplit-{k}",
                        engine=ins.engine,
                        ins=[],
                        outs=[],
                        sync_info=mybir.SyncInfo(on_wait=chunk, on_update=[]),
                    )
                    out.append(nop)
                    k += 1
                si.on_wait = keep
                ins.sync_info = si
                changed = True
                n_fixed += 1
            out.append(ins)
        if changed:
            try:
                bb.instructions = out
            except Exception:
                bb.instructions.clear()
                bb.instructions.extend(out)
    return n_fixed


# ---------------------------------------------------------------------------
# bass program
# ---------------------------------------------------------------------------
def _build_nc(T):
    import concourse.bass as bass
    import concourse.mybir as mybir
    from concourse.tile import TileContext

    dt = mybir.dt
    AF = mybir.ActivationFunctionType

    nc = bass.Bass()

    def din(name, shape, dtype=dt.bfloat16):
        return nc.dram_tensor(name, shape, dtype, kind="ExternalInput")

    # weights, pre-chunked on host into lhsT layout [128(k-rows), m, k, 128(cols)]
    wrz = din("wrz", [128, 4, KC, 128])          # merged (w_ih+w_hh)[:2H], steps>=2
    whr = din("whr", [128, 4, KC, 128])          # w_hh[:2H], step 1
    wni = din("wni", [128, 2, KC, 128])          # w_ih[2H:3H]
    wnh = din("wnh", [128, 2, KC, 128])          # w_hh[2H:3H]
    w1 = din("w1", [128, 2, KC, 128])            # fnn_w1
    w2 = din("w2", [128, 2, KC, 128])            # fnn_w2
    owt = din("owt", [128, KC, 128])             # out_w.T chunks
    embT = din("embT", [128, KC, BS])            # emb shard, host-transposed

    brz_l = din("brz_l", [4, 128])               # (b_ih+b_hh)[:2H] rows per m-tile
    bh_l = din("bh_l", [4, 128])                 # b_hh[:2H] rows (step 1)
    bni_v = din("bni_v", [128, 2], dt.float32)   # b_ih[2H:] per-partition cols
    bnh_v = din("bnh_v", [128, 2], dt.float32)   # b_hh[2H:] per-partition cols
    b1_l = din("b1_l", [2, 128])                 # fnn_b1
    b2_l = din("b2_l", [2, 128])                 # fnn_b2
    sel4 = din("sel4", [4, 4, BS])               # one-hot bias spreaders
    sel2 = din("sel2", [2, 2, BS])
    ones1 = din("ones1", [1, BS])
    oby = din("oby", [1, YB, M])                 # out_b tiled for the y bank

    y_out = nc.dram_tensor("y", [BS, T, M], dt.float32, kind="ExternalOutput")
    hx_out = nc.dram_tensor("hxT", [128, KC, BS], dt.float32, kind="ExternalOutput")

    R = int(os.environ.get("MNR_REPEAT", "1"))

    with TileContext(nc) as tc:
        with (
            tc.tile_pool(name="wpool", bufs=1) as wp,
            tc.tile_pool(name="state", bufs=3) as sp,
            tc.tile_pool(name="tmp", bufs=3) as tp,
            tc.tile_pool(name="ysb", bufs=3) as yp,
            tc.tile_pool(name="ps_rz", bufs=2, space="PSUM") as prz,
            tc.tile_pool(name="ps_hn", bufs=2, space="PSUM") as phn,
            tc.tile_pool(name="ps_in", bufs=2, space="PSUM") as pin,
            tc.tile_pool(name="ps_y", bufs=2, space="PSUM") as py,
        ):
            def load(dram, shape, tag):
                t = wp.tile(shape, dram.dtype, tag=tag)
                nc.sync.dma_start(t[:], dram[:])
                return t

            wrz_s = load(wrz, [128, 4, KC, 128], "wrz")
            whr_s = load(whr, [128, 4, KC, 128], "whr")
            wni_s = load(wni, [128, 2, KC, 128], "wni")
            wnh_s = load(wnh, [128, 2, KC, 128], "wnh")
            w1_s = load(w1, [128, 2, KC, 128], "w1")
            w2_s = load(w2, [128, 2, KC, 128], "w2")
            owt_s = load(owt, [128, KC, 128], "owt")
            embT_s = load(embT, [128, KC, BS], "embT")
            brz_s = load(brz_l, [4, 128], "brz")
            bh_s = load(bh_l, [4, 128], "bh")
            bni_s = load(bni_v, [128, 2], "bni")
            bnh_s = load(bnh_v, [128, 2], "bnh")
            b1_s = load(b1_l, [2, 128], "b1")
            b2_s = load(b2_l, [2, 128], "b2")
            sel4_s = load(sel4, [4, 4, BS], "sel4")
            sel2_s = load(sel2, [2, 2, BS], "sel2")
            ones_s = load(ones1, [1, BS], "ones1")
            oby_s = load(oby, [1, YB, M], "oby")

            MMs = nc.tensor.matmul

            def emit_body(_iv=None):
                # ---------------- FNN prologue ----------------
                # f1 = relu(W1 @ embT + b1)   (bias via one-hot K=2 matmul)
                f1_ps = prz.tile([128, 4, BS], dt.float32, tag="rzps")
                MMs(f1_ps[:, 0:2, :], b1_s[:], sel2_s[:], start=True, stop=False,
                    skip_group_check=True)
                for m in range(2):
                    for k in range(KC):
                        MMs(f1_ps[:, m, :], w1_s[:, m, k, :], embT_s[:, k, :],
                            start=False, stop=(k == KC - 1), skip_group_check=True)
                f1_sb = tp.tile([128, KC, BS], dt.bfloat16, tag="f1")
                nc.scalar.activation(f1_sb[:], f1_ps[:, 0:2, :], AF.Relu)

                # hx = W2 @ f1 + b2
                hx_ps = phn.tile([128, 2, BS], dt.float32, tag="hnps")
                MMs(hx_ps[:], b2_s[:], sel2_s[:], start=True, stop=False,
                    skip_group_check=True)
                for m in range(2):
                    for k in range(KC):
                        MMs(hx_ps[:, m, :], w2_s[:, m, k, :], f1_sb[:, k, :],
                            start=False, stop=(k == KC - 1), skip_group_check=True)
                h_prev = sp.tile([128, KC, BS], dt.bfloat16, tag="h")
                nc.scalar.activation(h_prev[:], hx_ps[:], AF.Copy)

                # ---------------- GRU scan ----------------
                y_ps = None
                for t in range(T):
                    first = t == 0
                    w_rz = whr_s if first else wrz_s
                    b_rz = bh_s if first else brz_s

                    # r/z gates: psum bank [128, (r0,r1,z0,z1), BS]
                    rz_ps = prz.tile([128, 4, BS], dt.float32, tag="rzps")
                    MMs(rz_ps[:], b_rz[:], sel4_s[:], start=True, stop=False,
                        skip_group_check=True)
                    # r tiles first (they head the critical chain)
                    for m in (0, 1, 2, 3):
                        for k in range(KC):
                            MMs(rz_ps[:, m, :], w_rz[:, m, k, :], h_prev[:, k, :],
                                start=False, stop=(k == KC - 1),
                                skip_group_check=True)
                        if m == 1:
                            # hn-path matmuls fill the PE while sigmoid(r) runs
                            hn_ps = phn.tile([128, 2, BS], dt.float32, tag="hnps")
                            MMs(hn_ps[:], bnh_s[:], sel2_s[:], start=True,
                                stop=False, skip_group_check=True)
                            for mm in range(2):
                                for k in range(KC):
                                    MMs(hn_ps[:, mm, :], wnh_s[:, mm, k, :],
                                        h_prev[:, k, :], start=False,
                                        stop=(k == KC - 1), skip_group_check=True)
                            r_sb = tp.tile([128, 2, BS], dt.bfloat16, tag="r")
                            nc.scalar.activation(r_sb[:], rz_ps[:, 0:2, :],
                                                 AF.Sigmoid)

                    # i_n path (bias always; matmuls only for steps >= 2)
                    in_ps = pin.tile([128, 2, BS], dt.float32, tag="inps")
                    MMs(in_ps[:], bni_s[:], sel2_s[:], start=True, stop=first,
                        skip_group_check=True)
                    if not first:
                        for mm in range(2):
                            for k in range(KC):
                                MMs(in_ps[:, mm, :], wni_s[:, mm, k, :],
                                    h_prev[:, k, :], start=False,
                                    stop=(k == KC - 1), skip_group_check=True)

                    z_sb = tp.tile([128, 2, BS], dt.bfloat16, tag="z")
                    nc.scalar.activation(z_sb[:], rz_ps[:, 2:4, :], AF.Sigmoid)

                    # n = tanh(i_n + r * h_n)
                    rhn = tp.tile([128, 2, BS], dt.float32, tag="rhn")
                    nc.vector.tensor_mul(rhn[:], hn_ps[:], r_sb[:])
                    npre = tp.tile([128, 2, BS], dt.float32, tag="npre")
                    nc.vector.tensor_add(npre[:], in_ps[:], rhn[:])
                    n_sb = tp.tile([128, 2, BS], dt.bfloat16, tag="n")
                    nc.scalar.activation(n_sb[:], npre[:], AF.Tanh)

                    # h' = n + z * (h - n)
                    hmn = tp.tile([128, 2, BS], dt.bfloat16, tag="hmn")
                    nc.vector.tensor_sub(hmn[:], h_prev[:], n_sb[:])
                    zm = tp.tile([128, 2, BS], dt.bfloat16, tag="zm")
                    nc.vector.tensor_mul(zm[:], z_sb[:], hmn[:])
                    h_new = sp.tile([128, KC, BS], dt.bfloat16, tag="h")
                    nc.vector.tensor_add(h_new[:], n_sb[:], zm[:])

                    # y_t = h' @ out_w.T + out_b  (batch-major psum [BS, M])
                    j = t % YB
                    if j == 0:
                        y_ps = py.tile([BS, YB, M], dt.float32, tag="yps")
                        MMs(y_ps[:], ones_s[:], oby_s[:], start=True, stop=False,
                            skip_group_check=True)
                    for k in range(KC):
                        MMs(y_ps[:, j, :], h_new[:, k, :], owt_s[:, k, :],
                            start=False, stop=(k == KC - 1), skip_group_check=True)
                    if j == YB - 1 or t == T - 1:
                        y_sb = yp.tile([BS, YB, M], dt.float32, tag="ysb")
                        nc.scalar.activation(y_sb[:, : j + 1, :],
                                             y_ps[:, : j + 1, :], AF.Copy)
                        t0 = t - j
                        nc.sync.dma_start(y_out[:, t0 : t + 1, :],
                                          y_sb[:, : j + 1, :])

                    h_prev = h_new

                # hx_final (still transposed; host untransposes)
                hxf = tp.tile([128, KC, BS], dt.float32, tag="hxf")
                nc.scalar.activation(hxf[:], h_prev[:], AF.Copy)
                nc.sync.dma_start(hx_out[:], hxf[:])

            if R == 1:
                emit_body()
            else:
                with tc.For_i(0, R, 1):
                    emit_body()

    _split_excess_waits(nc)
    return nc


# ---------------------------------------------------------------------------
# host side
# ---------------------------------------------------------------------------
def _chunk_lhsT(w, m_tiles):
    """[m_tiles*128, KC*128] weight -> lhsT chunks [128(k-rows), m, k, 128(m-cols)].

    lhsT for (m, k) must be W[m*128:(m+1)*128, k*128:(k+1)*128].T
    """
    m4 = w.reshape(m_tiles, 128, KC, 128)        # [m, c(row), k, p(col of chunk)]
    return np.ascontiguousarray(m4.transpose(3, 0, 2, 1)).astype(BF16)


def make_in_maps(emb, fnn_w1, fnn_b1, fnn_w2, fnn_b2,
                 gru_w_ih, gru_b_ih, gru_w_hh, gru_b_hh, out_w, out_b):
    emb = np.asarray(emb, np.float32)
    w_ih = np.asarray(gru_w_ih, np.float32)
    w_hh = np.asarray(gru_w_hh, np.float32)
    b_ih = np.asarray(gru_b_ih, np.float32)
    b_hh = np.asarray(gru_b_hh, np.float32)

    common = {
        "wrz": _chunk_lhsT(w_ih[: 2 * H] + w_hh[: 2 * H], 4),
        "whr": _chunk_lhsT(w_hh[: 2 * H], 4),
        "wni": _chunk_lhsT(w_ih[2 * H :], 2),
        "wnh": _chunk_lhsT(w_hh[2 * H :], 2),
        "w1": _chunk_lhsT(np.asarray(fnn_w1, np.float32), 2),
        "w2": _chunk_lhsT(np.asarray(fnn_w2, np.float32), 2),
        "owt": np.ascontiguousarray(
            np.asarray(out_w, np.float32).reshape(M, KC, 128).transpose(2, 1, 0)
        ).astype(BF16),
        "brz_l": (b_ih[: 2 * H] + b_hh[: 2 * H]).reshape(4, 128).astype(BF16),
        "bh_l": b_hh[: 2 * H].reshape(4, 128).astype(BF16),
        "bni_l": b_ih[2 * H :].reshape(2, 128).astype(BF16),
        "bnh_l": b_hh[2 * H :].reshape(2, 128).astype(BF16),
        "b1_l": np.asarray(fnn_b1, np.float32).reshape(2, 128).astype(BF16),
        "b2_l": np.asarray(fnn_b2, np.float32).reshape(2, 128).astype(BF16),
        "sel4": np.repeat(np.eye(4, dtype=np.float32)[:, :, None], BS, 2).astype(BF16),
        "sel2": np.repeat(np.eye(2, dtype=np.float32)[:, :, None], BS, 2).astype(BF16),
        "ones1": np.ones((1, BS), BF16),
        "oby": np.broadcast_to(
            np.asarray(out_b, np.float32), (1, YB, M)
        ).astype(BF16).copy(),
    }

    in_maps = []
    for c in range(NC):
        sh = emb[c * BS : (c + 1) * BS]                       # [BS, E]
        embT = np.ascontiguousarray(
            sh.T.reshape(KC, 128, BS).transpose(1, 0, 2)
        ).astype(BF16)
        m = dict(common)
        m["embT"] = embT
        in_maps.append(m)
    return in_maps


def assemble_outputs(results, T):
    y = np.empty((B, T, M), np.float32)
    hx = np.empty((B, H), np.float32)
    for c in range(NC):
        r = results[c]
        y[c * BS : (c + 1) * BS] = r["y"]
        hxT = r["hxT"]                                        # [128, KC, BS]
        hx[c * BS : (c + 1) * BS] = hxT.transpose(1, 0, 2).reshape(H, BS).T
    return y, hx


def kernel(emb, fnn_w1, fnn_b1, fnn_w2, fnn_b2,
           gru_w_ih, gru_b_ih, gru_w_hh, gru_b_hh, out_w, out_b):
    from concourse.bass_utils import run_bass_kernel_spmd

    T = int(os.environ.get("MNR_T", T_FULL))
    in_maps = make_in_maps(emb, fnn_w1, fnn_b1, fnn_w2, fnn_b2,
                           gru_w_ih, gru_b_ih, gru_w_hh, gru_b_hh, out_w, out_b)
    nc = _build_nc(T)
    res = run_bass_kernel_spmd(nc, in_maps, core_ids=list(range(NC)))
    kernel.last_result = res
    y, hx = assemble_outputs(res.results, T)
    return y, hx


# revision 8
# speedup vs baseline: 14.0764x; 1.0178x over previous
"""MetaNodeRNN Trainium2 kernel.

FNN (2-layer MLP) -> 512-step GRU scan -> output Linear, data-parallel over
batch across 8 NeuronCores (32 rows/core). GRU state is kept "transposed"
(H on partitions, batch on free dim) so every per-step tensor op uses all
128 partitions and no transposes are needed inside the scan.

Key tricks:
  - For steps >= 2 the GRU input equals the hidden state, so the r/z gate
    matmuls use merged weights (w_ih + w_hh)[:2H] -- one matmul instead of two.
  - Biases are injected into the PSUM accumulation groups via tiny one-hot
    matmuls (K=4 / K=2 / K=1), so activations never need bias vectors.
  - Weights are the stationary matmul operand (bf16 => fast weight load);
    the state (bf16) streams through.
  - y_t = h_t @ out_w.T + out_b is computed interleaved on the PE,
    accumulated 4 steps per PSUM bank, then copied to SBUF + DMA'd out.

MNR_T overrides the step count (testing); MNR_REPEAT wraps the whole
FNN+scan+output body in a hardware For_i loop (timing calibration).
"""

import os

import ml_dtypes
import numpy as np

B, E, H, M, T_FULL = 256, 256, 256, 128, 512
NC = 8
BS = B // NC          # 32 batch rows per core
KC = H // 128         # 2 contraction chunks of 128
YB = 4                # y steps accumulated per PSUM bank

BF16 = ml_dtypes.bfloat16


# ---------------------------------------------------------------------------
# walrus in this container rejects >1 semaphore wait per CTRL instruction;
# redistribute excess waits onto inserted Drain instructions.
# ---------------------------------------------------------------------------
def _split_excess_waits(nc, max_waits=1):
    import concourse.mybir as mybir

    n_fixed = 0
    for bb in nc.main_func.blocks:
        insts = list(bb.instructions)
        out = []
        changed = False
        for ins in insts:
            si = ins.sync_info
            if si is not None and si.on_wait and len(si.on_wait) > max_waits:
                waits = list(si.on_wait)
                extra, keep = waits[:-max_waits], waits[-max_waits:]
                k = 0
                while extra:
                    chunk, extra = extra[:max_waits], extra[max_waits:]
                    nop = mybir.InstDrain(
                        name=f"{ins.name}-waitsplit-{k}",
                        engine=ins.engine,
                        ins=[],
                        outs=[],
                        sync_info=mybir.SyncInfo(on_wait=chunk, on_update=[]),
                    )
                    out.append(nop)
                    k += 1
                si.on_wait = keep
                ins.sync_info = si
                changed = True
                n_fixed += 1
            out.append(ins)
        if changed:
            try:
                bb.instructions = out
            except Exception:
                bb.instructions.clear()
                bb.instructions.extend(out)
    return n_fixed


# ---------------------------------------------------------------------------
# bass program
# ---------------------------------------------------------------------------
def _build_nc(T):
    import concourse.bass as bass
    import concourse.mybir as mybir
    from concourse.tile import TileContext

    dt = mybir.dt
    AF = mybir.ActivationFunctionType

    nc = bass.Bass()

    def din(name, shape, dtype=dt.bfloat16):
        return nc.dram_tensor(name, shape, dtype, kind="ExternalInput")

    # weights, pre-chunked on host into lhsT layout [128(k-rows), m, k, 128(cols)]
    wrz = din("wrz", [128, 4, KC, 128])          # merged (w_ih+w_hh)[:2H], steps>=2
    whr = din("whr", [128, 4, KC, 128])          # w_hh[:2H], step 1
    wni = din("wni", [128, 2, KC, 128])          # w_ih[2H:3H]
    wnh = din("wnh", [128, 2, KC, 128])          # w_hh[2H:3H]
    w1 = din("w1", [128, 2, KC, 128])            # fnn_w1
    w2 = din("w2", [128, 2, KC, 128])            # fnn_w2
    owt = din("owt", [128, KC, 128])             # out_w.T chunks
    embT = din("embT", [128, KC, BS])            # emb shard, host-transposed

    brz_l = din("brz_l", [4, 128])               # (b_ih+b_hh)[:2H] rows per m-tile
    bh_l = din("bh_l", [4, 128])                 # b_hh[:2H] rows (step 1)
    bni_v = din("bni_v", [128, 2], dt.float32)   # b_ih[2H:] per-partition cols
    bnh_v = din("bnh_v", [128, 2], dt.float32)   # b_hh[2H:] per-partition cols
    b1_l = din("b1_l", [2, 128])                 # fnn_b1
    b2_l = din("b2_l", [2, 128])                 # fnn_b2
    sel4 = din("sel4", [4, 4, BS])               # one-hot bias spreaders
    sel2 = din("sel2", [2, 2, BS])
    ones1 = din("ones1", [1, BS])
    oby = din("oby", [1, YB, M])                 # out_b tiled for the y bank

    y_out = nc.dram_tensor("y", [BS, T, M], dt.float32, kind="ExternalOutput")
    hx_out = nc.dram_tensor("hxT", [128, KC, BS], dt.float32, kind="ExternalOutput")

    R = int(os.environ.get("MNR_REPEAT", "1"))

    with TileContext(nc) as tc:
        with (
            tc.tile_pool(name="wpool", bufs=1) as wp,
            tc.tile_pool(name="state", bufs=3) as sp,
            tc.tile_pool(name="tmp", bufs=3) as tp,
            tc.tile_pool(name="ysb", bufs=3) as yp,
            tc.tile_pool(name="ps_rz", bufs=2, space="PSUM") as prz,
            tc.tile_pool(name="ps_hn", bufs=2, space="PSUM") as phn,
            tc.tile_pool(name="ps_in", bufs=2, space="PSUM") as pin,
            tc.tile_pool(name="ps_y", bufs=2, space="PSUM") as py,
        ):
            def load(dram, shape, tag):
                t = wp.tile(shape, dram.dtype, tag=tag)
                nc.sync.dma_start(t[:], dram[:])
                return t

            wrz_s = load(wrz, [128, 4, KC, 128], "wrz")
            whr_s = load(whr, [128, 4, KC, 128], "whr")
            wni_s = load(wni, [128, 2, KC, 128], "wni")
            wnh_s = load(wnh, [128, 2, KC, 128], "wnh")
            w1_s = load(w1, [128, 2, KC, 128], "w1")
            w2_s = load(w2, [128, 2, KC, 128], "w2")
            owt_s = load(owt, [128, KC, 128], "owt")
            embT_s = load(embT, [128, KC, BS], "embT")
            brz_s = load(brz_l, [4, 128], "brz")
            bh_s = load(bh_l, [4, 128], "bh")
            bni_s = load(bni_v, [128, 2], "bni")
            bnh_s = load(bnh_v, [128, 2], "bnh")
            b1_s = load(b1_l, [2, 128], "b1")
            b2_s = load(b2_l, [2, 128], "b2")
            sel4_s = load(sel4, [4, 4, BS], "sel4")
            sel2_s = load(sel2, [2, 2, BS], "sel2")
            ones_s = load(ones1, [1, BS], "ones1")
            oby_s = load(oby, [1, YB, M], "oby")

            MMs = nc.tensor.matmul

            def emit_body(_iv=None):
                # ---------------- FNN prologue ----------------
                # f1 = relu(W1 @ embT + b1)   (bias via one-hot K=2 matmul)
                f1_ps = prz.tile([128, 4, BS], dt.float32, tag="rzps")
                MMs(f1_ps[:, 0:2, :], b1_s[:], sel2_s[:], start=True, stop=False,
                    skip_group_check=True)
                for m in range(2):
                    for k in range(KC):
                        MMs(f1_ps[:, m, :], w1_s[:, m, k, :], embT_s[:, k, :],
                            start=False, stop=(k == KC - 1), skip_group_check=True)
                f1_sb = tp.tile([128, KC, BS], dt.bfloat16, tag="f1")
                nc.scalar.activation(f1_sb[:], f1_ps[:, 0:2, :], AF.Relu)

                # hx = W2 @ f1 + b2
                hx_ps = phn.tile([128, 2, BS], dt.float32, tag="hnps")
                MMs(hx_ps[:], b2_s[:], sel2_s[:], start=True, stop=False,
                    skip_group_check=True)
                for m in range(2):
                    for k in range(KC):
                        MMs(hx_ps[:, m, :], w2_s[:, m, k, :], f1_sb[:, k, :],
                            start=False, stop=(k == KC - 1), skip_group_check=True)
                h_prev = sp.tile([128, KC, BS], dt.bfloat16, tag="h")
                nc.scalar.activation(h_prev[:], hx_ps[:], AF.Copy)

                # ---------------- GRU scan ----------------
                y_ps = None
                for t in range(T):
                    first = t == 0
                    w_rz = whr_s if first else wrz_s
                    b_rz = bh_s if first else brz_s

                    # r/z gates: psum bank [128, (r0,r1,z0,z1), BS]
                    rz_ps = prz.tile([128, 4, BS], dt.float32, tag="rzps")
                    MMs(rz_ps[:], b_rz[:], sel4_s[:], start=True, stop=False,
                        skip_group_check=True)
                    # r tiles first (they head the critical chain)
                    for m in (0, 1, 2, 3):
                        for k in range(KC):
                            MMs(rz_ps[:, m, :], w_rz[:, m, k, :], h_prev[:, k, :],
                                start=False, stop=(k == KC - 1),
                                skip_group_check=True)
                        if m == 1:
                            # hn-path matmuls fill the PE while sigmoid(r) runs
                            hn_ps = phn.tile([128, 2, BS], dt.float32, tag="hnps")
                            for mm in range(2):
                                for k in range(KC):
                                    MMs(hn_ps[:, mm, :], wnh_s[:, mm, k, :],
                                        h_prev[:, k, :], start=(k == 0),
                                        stop=(k == KC - 1), skip_group_check=True)
                            r_sb = tp.tile([128, 2, BS], dt.bfloat16, tag="r")
                            nc.scalar.activation(r_sb[:], rz_ps[:, 0:2, :],
                                                 AF.Sigmoid)

                    # i_n path (matmuls only for steps >= 2; bias folded in stt)
                    in_ps = None
                    if not first:
                        in_ps = pin.tile([128, 2, BS], dt.float32, tag="inps")
                        for mm in range(2):
                            for k in range(KC):
                                MMs(in_ps[:, mm, :], wni_s[:, mm, k, :],
                                    h_prev[:, k, :], start=(k == 0),
                                    stop=(k == KC - 1), skip_group_check=True)

                    z_sb = tp.tile([128, 2, BS], dt.bfloat16, tag="z")
                    nc.scalar.activation(z_sb[:], rz_ps[:, 2:4, :], AF.Sigmoid)

                    # n = tanh(i_n + b_ni + r * (h_n + b_nh))
                    rhn = tp.tile([128, 2, BS], dt.float32, tag="rhn")
                    for c in range(KC):
                        nc.vector.scalar_tensor_tensor(
                            rhn[:, c, :], hn_ps[:, c, :], bnh_s[:, c : c + 1],
                            r_sb[:, c, :], mybir.AluOpType.add,
                            mybir.AluOpType.mult)
                    npre = tp.tile([128, 2, BS], dt.float32, tag="npre")
                    for c in range(KC):
                        if first:
                            nc.vector.tensor_scalar_add(
                                npre[:, c, :], rhn[:, c, :], bni_s[:, c : c + 1])
                        else:
                            nc.vector.scalar_tensor_tensor(
                                npre[:, c, :], in_ps[:, c, :], bni_s[:, c : c + 1],
                                rhn[:, c, :], mybir.AluOpType.add,
                                mybir.AluOpType.add)
                    n_sb = tp.tile([128, 2, BS], dt.bfloat16, tag="n")
                    nc.scalar.activation(n_sb[:], npre[:], AF.Tanh)

                    # h' = n + z * (h - n)
                    hmn = tp.tile([128, 2, BS], dt.bfloat16, tag="hmn")
                    nc.vector.tensor_sub(hmn[:], h_prev[:], n_sb[:])
                    zm = tp.tile([128, 2, BS], dt.bfloat16, tag="zm")
                    nc.vector.tensor_mul(zm[:], z_sb[:], hmn[:])
                    h_new = sp.tile([128, KC, BS], dt.bfloat16, tag="h")
                    nc.vector.tensor_add(h_new[:], n_sb[:], zm[:])

                    # y_t = h' @ out_w.T + out_b  (batch-major psum [BS, M])
                    j = t % YB
                    if j == 0:
                        y_ps = py.tile([BS, YB, M], dt.float32, tag="yps")
                        MMs(y_ps[:], ones_s[:], oby_s[:], start=True, stop=False,
                            skip_group_check=True)
                    for k in range(KC):
                        MMs(y_ps[:, j, :], h_new[:, k, :], owt_s[:, k, :],
                            start=False, stop=(k == KC - 1), skip_group_check=True)
                    if j == YB - 1 or t == T - 1:
                        y_sb = yp.tile([BS, YB, M], dt.float32, tag="ysb")
                        nc.scalar.activation(y_sb[:, : j + 1, :],
                                             y_ps[:, : j + 1, :], AF.Copy)
                        t0 = t - j
                        nc.sync.dma_start(y_out[:, t0 : t + 1, :],
                                          y_sb[:, : j + 1, :])

                    h_prev = h_new

                # hx_final (still transposed; host untransposes)
                hxf = tp.tile([128, KC, BS], dt.float32, tag="hxf")
                nc.scalar.activation(hxf[:], h_prev[:], AF.Copy)
                nc.sync.dma_start(hx_out[:], hxf[:])

            if R == 1:
                emit_body()
            else:
                with tc.For_i(0, R, 1):
                    emit_body()

    _split_excess_waits(nc)
    return nc


# ---------------------------------------------------------------------------
# host side
# ---------------------------------------------------------------------------
def _chunk_lhsT(w, m_tiles):
    """[m_tiles*128, KC*128] weight -> lhsT chunks [128(k-rows), m, k, 128(m-cols)].

    lhsT for (m, k) must be W[m*128:(m+1)*128, k*128:(k+1)*128].T
    """
    m4 = w.reshape(m_tiles, 128, KC, 128)        # [m, c(row), k, p(col of chunk)]
    return np.ascontiguousarray(m4.transpose(3, 0, 2, 1)).astype(BF16)


def make_in_maps(emb, fnn_w1, fnn_b1, fnn_w2, fnn_b2,
                 gru_w_ih, gru_b_ih, gru_w_hh, gru_b_hh, out_w, out_b):
    emb = np.asarray(emb, np.float32)
    w_ih = np.asarray(gru_w_ih, np.float32)
    w_hh = np.asarray(gru_w_hh, np.float32)
    b_ih = np.asarray(gru_b_ih, np.float32)
    b_hh = np.asarray(gru_b_hh, np.float32)

    common = {
        "wrz": _chunk_lhsT(w_ih[: 2 * H] + w_hh[: 2 * H], 4),
        "whr": _chunk_lhsT(w_hh[: 2 * H], 4),
        "wni": _chunk_lhsT(w_ih[2 * H :], 2),
        "wnh": _chunk_lhsT(w_hh[2 * H :], 2),
        "w1": _chunk_lhsT(np.asarray(fnn_w1, np.float32), 2),
        "w2": _chunk_lhsT(np.asarray(fnn_w2, np.float32), 2),
        "owt": np.ascontiguousarray(
            np.asarray(out_w, np.float32).reshape(M, KC, 128).transpose(2, 1, 0)
        ).astype(BF16),
        "brz_l": (b_ih[: 2 * H] + b_hh[: 2 * H]).reshape(4, 128).astype(BF16),
        "bh_l": b_hh[: 2 * H].reshape(4, 128).astype(BF16),
        "bni_v": np.ascontiguousarray(b_ih[2 * H :].reshape(2, 128).T),
        "bnh_v": np.ascontiguousarray(b_hh[2 * H :].reshape(2, 128).T),
        "b1_l": np.asarray(fnn_b1, np.float32).reshape(2, 128).astype(BF16),
        "b2_l": np.asarray(fnn_b2, np.float32).reshape(2, 128).astype(BF16),
        "sel4": np.repeat(np.eye(4, dtype=np.float32)[:, :, None], BS, 2).astype(BF16),
        "sel2": np.repeat(np.eye(2, dtype=np.float32)[:, :, None], BS, 2).astype(BF16),
        "ones1": np.ones((1, BS), BF16),
        "oby": np.broadcast_to(
            np.asarray(out_b, np.float32), (1, YB, M)
        ).astype(BF16).copy(),
    }

    in_maps = []
    for c in range(NC):
        sh = emb[c * BS : (c + 1) * BS]                       # [BS, E]
        embT = np.ascontiguousarray(
            sh.T.reshape(KC, 128, BS).transpose(1, 0, 2)
        ).astype(BF16)
        m = dict(common)
        m["embT"] = embT
        in_maps.append(m)
    return in_maps


def assemble_outputs(results, T):
    y = np.empty((B, T, M), np.float32)
    hx = np.empty((B, H), np.float32)
    for c in range(NC):
        r = results[c]
        y[c * BS : (c + 1) * BS] = r["y"]
        hxT = r["hxT"]                                        # [128, KC, BS]
        hx[c * BS : (c + 1) * BS] = hxT.transpose(1, 0, 2).reshape(H, BS).T
    return y, hx


def kernel(emb, fnn_w1, fnn_b1, fnn_w2, fnn_b2,
           gru_w_ih, gru_b_ih, gru_w_hh, gru_b_hh, out_w, out_b):
    from concourse.bass_utils import run_bass_kernel_spmd

    T = int(os.environ.get("MNR_T", T_FULL))
    in_maps = make_in_maps(emb, fnn_w1, fnn_b1, fnn_w2, fnn_b2,
                           gru_w_ih, gru_b_ih, gru_w_hh, gru_b_hh, out_w, out_b)
    nc = _build_nc(T)
    res = run_bass_kernel_spmd(nc, in_maps, core_ids=list(range(NC)))
    kernel.last_result = res
    y, hx = assemble_outputs(res.results, T)
    return y, hx
